# revision 42
# baseline (speedup 1.0000x reference)
"""Trainium2 Bass kernel for nn_CapgMyoNet (dense CNN), 8-core data-parallel.

V2: position-parity layout redesign.
- A2 (conv1 out): partitions [0:64]=ch of even-x positions, [64:128]=ch of
  odd-x positions; cols = (y, xpair j, sample). All evictions dense 128-lane.
- conv1: M=128 matmuls (one per position pair), psum holds 2 pairs.
- conv2: K=128 matmuls using the parity layout; even-x outputs accumulate in
  psum[0:64] (tile (0,0)), odd-x in psum[64:128] (tile (0,64)); 12 weight
  mats (A/B even-class, C/D odd-class per ky).
- lc3/lc4: block-diagonal pixel-pair weights, K=128 M=128.
- fc5 accumulated on the fly; fc6/7/8 K-chunked as before.
All bn folded host-side; bf16 matmuls, fp32 accumulate.
"""
import numpy as np
import ml_dtypes

import concourse.bass as bass
import concourse.bacc as bacc
import concourse.mybir as mybir
import concourse.tile as tile

bf16 = mybir.dt.bfloat16
f32 = mybir.dt.float32
fp8 = mybir.dt.float8e4
DRSW = mybir.MatmulPerfMode.DoubleRowSwInterleave
e4m3 = ml_dtypes.float8_e4m3fn

H, W, C, NCLS = 8, 16, 64, 8
NPOS = H * W  # 128
NPAIR = 64    # 8 y rows x 8 x-pairs
EPS = 1e-5
NCORES = 8
PASSN = 256   # samples per conv pass
GROUP = 512   # samples per lc/fc5 group


def build(NB=1024, debug_taps=False):
    """Build the per-core bass program. NB = samples per core."""
    assert NB % GROUP == 0 and GROUP == 2 * PASSN
    NPASS = NB // PASSN
    NG = NB // GROUP
    NSPL = max(1, NB // 512)
    NCOLS = NB // NSPL

    nc = bacc.Bacc("TRN2", target_bir_lowering=False, debug=False)
    ev_ct = [0]

    def evict_split(out_ap, in_ap, bias_ap, n):
        """Latency-critical eviction: both halves in parallel on both engines."""
        h = n // 2
        nc.vector.tensor_scalar(out_ap[:, 0:h], in_ap[:, 0:h], bias_ap, 0.0,
                                mybir.AluOpType.add, mybir.AluOpType.max)
        nc.scalar.activation(out_ap[:, h:n], in_ap[:, h:n],
                             mybir.ActivationFunctionType.Relu, bias=bias_ap)

    def evict(out_ap, in_ap, bias_ap, relu=True):
        """Alternating-engine psum->sbuf eviction with bias (+relu)."""
        ev_ct[0] += 1
        if ev_ct[0] % 2 == 0:
            if relu:
                nc.scalar.activation(out_ap, in_ap,
                                     mybir.ActivationFunctionType.Relu,
                                     bias=bias_ap)
            else:
                nc.vector.tensor_scalar(out_ap, in_ap, bias_ap, None,
                                        mybir.AluOpType.add)
        else:
            if relu:
                nc.vector.tensor_scalar(out_ap, in_ap, bias_ap, 0.0,
                                        mybir.AluOpType.add,
                                        mybir.AluOpType.max)
            else:
                nc.vector.tensor_scalar(out_ap, in_ap, bias_ap, None,
                                        mybir.AluOpType.add)

    with tile.TileContext(nc) as tc:
        from contextlib import ExitStack
        es = ExitStack()
        with es:
            dram = es.enter_context(tc.tile_pool(name="dram", bufs=1, space="DRAM"))
            wp = es.enter_context(tc.tile_pool(name="wp", bufs=1))
            big = es.enter_context(tc.tile_pool(name="big", bufs=1))
            ring = es.enter_context(tc.tile_pool(name="ring", bufs=4))
            lcsb = es.enter_context(tc.tile_pool(name="lcsb", bufs=4))

            # ---------------- DRAM I/O ----------------
            boot_d = dram.tile([128, NB + 32 + 1024], bf16, kind="ExternalInput", name="boot", uniquify=False)
            m1p_d = dram.tile([128, NPAIR * 128], bf16, kind="ExternalInput", name="m1p", uniquify=False)
            w2n_d = dram.tile([128, 6 * 256], fp8, kind="ExternalInput", name="w2n", uniquify=False)
            w3bd_d = dram.tile([128, NPAIR * 128], bf16, kind="ExternalInput", name="w3bd", uniquify=False)
            w4bd_d = dram.tile([128, NPAIR * 128], bf16, kind="ExternalInput", name="w4bd", uniquify=False)
            fc5w_d = dram.tile([128, NPAIR * 512], bf16, kind="ExternalInput", name="fc5w", uniquify=False)
            fc6w_d = dram.tile([128, 16 * 128], bf16, kind="ExternalInput", name="fc6w", uniquify=False)
            fc78w_d = dram.tile([128, 4 * 128 + NCLS], bf16, kind="ExternalInput", name="fc78w", uniquify=False)

            y_d = dram.tile([NCLS, NB], f32, kind="ExternalOutput", name="y", uniquify=False)
            if debug_taps:
                dbg_a2_d = dram.tile([128, NPAIR * PASSN], f32, kind="ExternalOutput", name="dbg_a2", uniquify=False)
                dbg_a3_d = dram.tile([128, NPAIR * GROUP], f32, kind="ExternalOutput", name="dbg_a3", uniquify=False)
                dbg_f6_d = dram.tile([128, 4 * NB], f32, kind="ExternalOutput", name="dbg_f6", uniquify=False)

            # ---------------- persistent SBUF ----------------
            boot = wp.tile([128, NB + 32 + 1024], bf16, name="boot_sb")
            consts = boot[:, NB:NB + 32].bitcast(f32)
            B1 = consts[:, 0:1]; B2 = consts[:, 1:2]
            B3 = consts[:, 2:3]; B4 = consts[:, 3:4]
            B5 = [consts[:, 4 + m:5 + m] for m in range(4)]
            B6 = [consts[:, 8 + m:9 + m] for m in range(4)]
            B7 = consts[:, 12:13]
            S0 = consts[:, 13:14]; T0 = consts[:, 14:15]
            B8 = consts[0:NCLS, 15:16]

            w2n_sb = wp.tile([128, 6 * 256], fp8, name="w2n_sb")
            m1p_sb = wp.tile([128, NPAIR * 128 - 1024], bf16, name="m1p_sb")

            def m1pv(pr):
                if pr < 8:
                    return boot[:, NB + 32 + 128 * pr:NB + 32 + 128 * pr + 128]
                return m1p_sb[:, 128 * pr - 1024:128 * pr - 896]
            w3bd_sb = wp.tile([128, NPAIR * 128], bf16, name="w3bd_sb")
            w4bd_sb = wp.tile([128, NPAIR * 128], bf16, name="w4bd_sb")
            fc6w_sb = wp.tile([128, 16 * 128], bf16, name="fc6w_sb")
            fc78w_sb = wp.tile([128, 4 * 128 + NCLS], bf16, name="fc78w_sb")

            F6 = wp.tile([128, 4 * NB], bf16, name="F6")
            # A2/A3 split into quarter buffers (2 y-rows each) so consumers
            # wait on as few producer evictions as possible
            A2h = [big.tile([128, NPAIR * PASSN // 4], fp8, name=f"A2h{h}")
                   for h in range(4)]
            A3h = [big.tile([128, NPAIR * GROUP // 4], bf16, name=f"A3h{h}")
                   for h in range(4)]

            # ---------------- input/weight DMA issue ----------------
            # one "boot" DMA carries everything the first conv1 matmuls and
            # evicts need (xts, consts, m1p pairs 0-7): per-issue queue cost
            # is ~1.6us, so folding the three first-use loads into one issue
            # moves the first matmul ~5us earlier.  Everything else trails in
            # first-use order.  (issuing from scalar/gpsimd queues instead
            # was measured slower: it delays those engines' first evictions.)
            nc.sync.dma_start(out=boot[:], in_=boot_d[:])
            for q in range(4):
                a, b = 1024 + 1792 * q, min(1024 + 1792 * (q + 1), 8192)
                nc.sync.dma_start(out=m1p_sb[:, a - 1024:b - 1024],
                                  in_=m1p_d[:, a:b])
            nc.sync.dma_start(out=w2n_sb[:], in_=w2n_d[:])
            nc.sync.dma_start(out=w3bd_sb[:], in_=w3bd_d[:])
            nc.sync.dma_start(out=w4bd_sb[:], in_=w4bd_d[:])
            nc.sync.dma_start(out=fc6w_sb[:], in_=fc6w_d[:])
            nc.sync.dma_start(out=fc78w_sb[:], in_=fc78w_d[:])

            def xtsv(pi):
                return boot[:, PASSN * pi:PASSN * (pi + 1)]

            # A2 views: cols = (pair pr, s), pr = y*8 + j; quarters of 2 y-rows
            a2vh = [A2h[h][:].rearrange("q (y j s) -> q y j s", y=2, j=8, s=PASSN)
                    for h in range(4)]

            def a2row(r):
                return a2vh[r // 2][:, r % 2]
            a3vh = [A3h[h][:].rearrange("q (pr c) -> q pr c", pr=NPAIR // 4, c=GROUP)
                    for h in range(4)]
            f6v = F6[:].rearrange("q (m n) -> q m n", m=4, n=NB)
            F7 = wp.tile([128, 4 * NB], bf16, name="F7")
            F8 = wp.tile([128, NB], bf16, name="F8")
            y_sb = wp.tile([NCLS, NB], f32, name="y_sb")
            f7v = F7[:].rearrange("q (m n) -> q m n", m=4, n=NB)

            def fc6_stage(m, n0, w, pool):
                ps6 = pool.tile([128, 2 * PASSN], f32, name="psF", tag="psC2", bufs=4)
                for jj in range(4):
                    nc.tensor.matmul(ps6[:, 0:w],
                                     fc6w_sb[:, (4 * jj + m) * 128:(4 * jj + m) * 128 + 128],
                                     f6v[:, jj, n0:n0 + w],
                                     start=(jj == 0), stop=(jj == 3),
                                     skip_group_check=True)
                evict_split(f7v[:, m, n0:n0 + w], ps6[:, 0:w], B6[m], w)

            def fc7_stage(n0, w, pool):
                ps7 = pool.tile([128, 2 * PASSN], f32, name="psF", tag="psC2", bufs=4)
                for jj in range(4):
                    nc.tensor.matmul(ps7[:, 0:w],
                                     fc78w_sb[:, 128 * jj:128 * jj + 128],
                                     f7v[:, jj, n0:n0 + w],
                                     start=(jj == 0), stop=(jj == 3),
                                     skip_group_check=True)
                evict_split(F8[:, n0:n0 + w], ps7[:, 0:w], B7, w)

            def fc8_stage(n0, w, pool):
                ps8 = pool.tile([128, 2 * PASSN], f32, name="psF", tag="psC2", bufs=4)
                nc.tensor.matmul(ps8[0:NCLS, 0:w], fc78w_sb[:, 512:512 + NCLS],
                                 F8[:, n0:n0 + w], start=True, stop=True,
                                 skip_group_check=True)
                nc.vector.tensor_scalar(y_sb[:, n0:n0 + w], ps8[0:NCLS, 0:w], B8, None,
                                        mybir.AluOpType.add)
                if n0 + w == NB - 256:
                    nc.sync.dma_start(out=y_d[:, 0:n0 + w], in_=y_sb[:, 0:n0 + w])
                elif n0 + w == NB:
                    nc.sync.dma_start(out=y_d[:, n0:NB], in_=y_sb[:, n0:NB])

            def fc678(n, pool):
                """fc6/7/8 for one 512-sample group, split into half-group
                chunks so the serial evict->mm chains of the two halves
                overlap (matters for the final group = kernel tail)."""
                for h in range(2):
                    n0 = n * GROUP + h * (GROUP // 2)
                    for m in range(4):
                        fc6_stage(m, n0, GROUP // 2, pool)
                    fc7_stage(n0, GROUP // 2, pool)
                    fc8_stage(n0, GROUP // 2, pool)

            for g in range(NG):
                # ============ conv passes (2 per group) ============
                # two pools so the psC1 pool can open on the 4 lcq banks the
                # previous group's lc phase frees ~3.4us before its ps5 banks
                # (the fc5 drain + F6 evictions) — kills the ~2us group gap
                c1pp_cm = tc.tile_pool(name=f"c1ps{g}", bufs=1, space="PSUM")
                c1pp_pool = c1pp_cm.__enter__()
                c2pp_cm = tc.tile_pool(name=f"c2ps{g}", bufs=1, space="PSUM")
                c2pp_pool = c2pp_cm.__enter__()
                for pb in range(2):
                    pi = 2 * g + pb
                    nb0 = pi * PASSN
                    c0 = pb * PASSN  # col offset within the group for A3
                    # ---- conv1: one M=128 matmul per position pair; psum
                    # tiles span 2 banks (4 pairs) so evictions amortize the
                    # ~120-cycle per-instruction PSUM-read bubble ----
                    if True:
                        c1pp = c1pp_pool
                        for t2 in range(NPAIR // 4):
                            psC1 = c1pp.tile([128, 4 * PASSN], f32, name="psC1", tag="psC1", bufs=2)
                            for sub in range(4):
                                pr = 4 * t2 + sub
                                nc.tensor.matmul(
                                    psC1[:, sub * PASSN:(sub + 1) * PASSN],
                                    m1pv(pr),
                                    xtsv(pi),
                                    start=True, stop=True,
                                    skip_group_check=True)
                            a2dst = A2h[t2 // 4][:, 4 * PASSN * (t2 % 4):4 * PASSN * (t2 % 4 + 1)]
                            if t2 % 16 in (0, 2, 4, 6, 8, 10, 11, 13, 15):
                                nc.scalar.activation(a2dst, psC1[:, :],
                                                     mybir.ActivationFunctionType.Relu,
                                                     bias=B1)
                            else:
                                nc.vector.tensor_scalar(a2dst, psC1[:, :], B1, 0.0,
                                                        mybir.AluOpType.add,
                                                        mybir.AluOpType.max)
                            # group-0 fc6/7/8 spread across pass-2's conv1 so
                            # its psum->sbuf chain hides under conv1 matmuls
                            if pi == 2 and t2 in (1, 3, 5, 9):
                                fc6_stage({1: 0, 3: 1, 5: 2, 9: 3}[t2], 0, GROUP, c2pp_pool)
                            elif pi == 2 and t2 == 12:
                                fc7_stage(0, GROUP, c2pp_pool)
                            elif pi == 2 and t2 == 15:
                                fc8_stage(0, GROUP, c2pp_pool)
                            # (group-0 stages use full-width chunks)
                    # ---- conv2: fp8 DoubleRow (K=256 = two x-pairs) into A3 ----
                    # out pair j' = (cols 2j'+1, 2j'+2 mod 16); slots (pair j',
                    # pair j'+1) except j'=7 which uses slots (pair 0, pair 7)
                    # with a swapped wrap stationary.
                    w2v = w2n_sb[:].rearrange("p (q two m) -> p q two m",
                                              q=6, two=2, m=128)
                    if True:
                        c2pp = c2pp_pool
                        for yp in range(H):
                            kys = [ky for ky in range(3) if 0 <= yp + ky - 1 < H]
                            for jg in range(4):
                                psC2 = c2pp.tile([128, 512], f32, name="psC2", tag="psC2", bufs=4)
                                for u in range(2):
                                    jp = 2 * jg + u
                                    for i, ky in enumerate(kys):
                                        r = yp + ky - 1
                                        if jp < 7:
                                            rhs = a2row(r)[:, jp:jp + 2, :]
                                            lhsT = w2v[:, ky]
                                        else:
                                            rhs = a2row(r)[:, 0:8:7, :]
                                            lhsT = w2v[:, 3 + ky]
                                        nc.tensor.matmul(
                                            psC2[:, 256 * u:256 * u + 256],
                                            lhsT, rhs,
                                            start=(i == 0), stop=(i == len(kys) - 1),
                                            perf_mode=DRSW, skip_group_check=True)
                                ypl = yp % 2
                                evict(a3vh[yp // 2][:, 8 * ypl + 2 * jg:8 * ypl + 2 * jg + 2,
                                                    c0:c0 + PASSN],
                                      psC2[:].rearrange("q (j s) -> q j s", j=2, s=PASSN), B2)
                    if debug_taps and pi == NPASS - 1:
                        dbgt = wp.tile([128, NPAIR * PASSN], f32, name="dbg_a2sb")
                        nc.any.tensor_copy(dbgt[:, 0:NPAIR * PASSN // 2], A2h[0][:])
                        nc.sync.dma_start(out=dbg_a2_d[:], in_=dbgt[:])

                if debug_taps and g == 0:
                    dbgt3 = wp.tile([128, NPAIR * GROUP], f32, name="dbg_a3sb")
                    nc.any.tensor_copy(dbgt3[:], A3[:])
                    nc.sync.dma_start(out=dbg_a3_d[:], in_=dbgt3[:])

                c2pp_cm.__exit__(None, None, None)
                c1pp_cm.__exit__(None, None, None)
                # ============ lc3 + lc4 + fc5 over pixel pairs ============
                # lc3/lc4 run as 64x64 4-tile quads over pair-PAIRS (A=2P,
                # B=2P+1): pair A on diagonal tiles (0,0)/(64,64), pair B on
                # crossed tiles (0,64)/(64,0).  tmpB carries a swapped layout
                # (po-ch on partitions 0:64); lc4 un-swaps via crossed tiles
                # (w4bd has swapped blocks for odd pairs host-side), so F is
                # natural for fc5.  Slot pipeline keeps the two quads adjacent
                # (2 PE tiling-mode switches per slot, ~105ns each).
                with tc.tile_pool(name=f"lcps{g}", bufs=1, space="PSUM") as lcpp:
                    # ps5 ring created AFTER the first quad's lcq tags so the
                    # ps5 accumulators (whose F6 evictions finish last) land on
                    # the HIGH banks: the next group's psC1 pool then reuses
                    # the early-freed lcq banks instead of waiting ~2us.
                    ps5 = []
                    tmps = {}
                    fjs = {}
                    wsts = {}

                    def fetch_fc5w(k):
                        wst = ring.tile([128, 2048], bf16, name="wst", tag="wst")
                        nc.sync.dma_start(out=wst[:], in_=fc5w_d[:, 2048 * k:2048 * (k + 1)])
                        wsts[k] = wst

                    def lc3_quad(P):
                        if P % 2 == 0 and P // 2 + 2 < NPAIR // 4:
                            fetch_fc5w(P // 2 + 2)
                        A, B = 2 * P, 2 * P + 1
                        aA = a3vh[A // 16][:, A % 16, :]
                        aB = a3vh[B // 16][:, B % 16, :]
                        psA = lcpp.tile([128, GROUP], f32, name="psA", tag="lcq3a")
                        psB = lcpp.tile([128, GROUP], f32, name="psB", tag="lcq3b")
                        nc.tensor.matmul(psA[0:64, :], w3bd_sb[0:64, 128 * A:128 * A + 64],
                                         aA[0:64], start=True, stop=True,
                                         tile_position=(0, 0), skip_group_check=True)
                        nc.tensor.matmul(psA[64:128, :], w3bd_sb[64:128, 128 * A + 64:128 * A + 128],
                                         aA[64:128], start=True, stop=True,
                                         tile_position=(64, 64), skip_group_check=True)
                        nc.tensor.matmul(psB[64:128, :], w3bd_sb[0:64, 128 * B:128 * B + 64],
                                         aB[0:64], start=True, stop=True,
                                         tile_position=(0, 64), skip_group_check=True)
                        nc.tensor.matmul(psB[0:64, :], w3bd_sb[64:128, 128 * B + 64:128 * B + 128],
                                         aB[64:128], start=True, stop=True,
                                         tile_position=(64, 0), skip_group_check=True)
                        tA = lcsb.tile([128, GROUP], bf16, name="tA", tag="tmp")
                        tB = lcsb.tile([128, GROUP], bf16, name="tB", tag="tmp")
                        evict(tA[:, :], psA[:, :], B3)
                        evict(tB[:, :], psB[:, :], B3)
                        tmps[A], tmps[B] = tA, tB

                    def lc4_quad(P):
                        A, B = 2 * P, 2 * P + 1
                        tA, tB = tmps.pop(A), tmps.pop(B)
                        psA = lcpp.tile([128, GROUP], f32, name="psA4", tag="lcq4a")
                        psB = lcpp.tile([128, GROUP], f32, name="psB4", tag="lcq4b")
                        nc.tensor.matmul(psA[0:64, :], w4bd_sb[0:64, 128 * A:128 * A + 64],
                                         tA[0:64, :], start=True, stop=True,
                                         tile_position=(0, 0), skip_group_check=True)
                        nc.tensor.matmul(psA[64:128, :], w4bd_sb[64:128, 128 * A + 64:128 * A + 128],
                                         tA[64:128, :], start=True, stop=True,
                                         tile_position=(64, 64), skip_group_check=True)
                        # pair B: tmpB swapped in, crossed tiles unswap the out
                        nc.tensor.matmul(psB[64:128, :], w4bd_sb[0:64, 128 * B:128 * B + 64],
                                         tB[0:64, :], start=True, stop=True,
                                         tile_position=(0, 64), skip_group_check=True)
                        nc.tensor.matmul(psB[0:64, :], w4bd_sb[64:128, 128 * B + 64:128 * B + 128],
                                         tB[64:128, :], start=True, stop=True,
                                         tile_position=(64, 0), skip_group_check=True)
                        FA = lcsb.tile([128, GROUP], bf16, name="FA", tag="Fj")
                        FB = lcsb.tile([128, GROUP], bf16, name="FB", tag="Fj")
                        evict(FA[:, :], psA[:, :], B4)
                        evict(FB[:, :], psB[:, :], B4)
                        fjs[A], fjs[B] = FA, FB

                    def fc5_stage(j):
                        Fj = fjs.pop(j)
                        wst = wsts[j // 4] if j % 4 != 3 else wsts.pop(j // 4)
                        for m in range(4):
                            nc.tensor.matmul(ps5[m][:, :],
                                             wst[:, 512 * (j % 4) + 128 * m:512 * (j % 4) + 128 * m + 128],
                                             Fj[:, :], start=(j == 0), stop=(j == NPAIR - 1))

                    fetch_fc5w(0); fetch_fc5w(1)
                    NQ = NPAIR // 2
                    # slot pipeline: [lc4(s-1), lc3(s)] adjacent in 64x64
                    # mode (2 switches per slot), then 2 fc5 stages.
                    lc3_quad(0)
                    ps5.extend(lcpp.tile([128, GROUP], f32, name=f"ps5_{m}", tag=f"ps5_{m}")
                               for m in range(4))
                    for s in range(1, NQ + 2):
                        if s - 1 < NQ:
                            lc4_quad(s - 1)
                        if s < NQ:
                            lc3_quad(s)
                        if s - 2 >= 0:
                            fc5_stage(2 * (s - 2))
                            fc5_stage(2 * (s - 2) + 1)
                    for m in range(4):
                        evict_split(f6v[:, m, g * GROUP:(g + 1) * GROUP],
                                    ps5[m][:, :], B5[m], GROUP)

            # ============ fc6 / fc7 / fc8 for the last group ============
            if debug_taps:
                dbg6 = wp.tile([128, 4 * NB], f32, name="dbg_f6sb")
                nc.any.tensor_copy(dbg6[:], F6[:])
                nc.sync.dma_start(out=dbg_f6_d[:], in_=dbg6[:])

            with tc.tile_pool(name="fcps", bufs=3, space="PSUM") as fcpp:
                fc678(NG - 1, fcpp)

    nc.compile()
    return nc


# ---------------------------------------------------------------------------
# host-side weight preparation
# ---------------------------------------------------------------------------

def _bn_affine(p):
    g, b, m, v = p[0], p[1], p[2], p[3]
    s = g / np.sqrt(v + EPS)
    return s.astype(np.float32), (b - m * s).astype(np.float32)


def prep_weights(inputs, cast=True):
    bf = ml_dtypes.bfloat16 if cast else np.float32
    s0, t0 = _bn_affine(inputs['bn0']); s0, t0 = float(s0[0]), float(t0[0])
    s1, t1 = _bn_affine(inputs['bn1'])
    s2, t2 = _bn_affine(inputs['bn2'])
    s3, t3 = _bn_affine(inputs['bn3'])
    s4, t4 = _bn_affine(inputs['bn4'])
    s5, t5 = _bn_affine(inputs['bn5']); s5, t5 = float(s5[0]), float(t5[0])
    s6, t6 = _bn_affine(inputs['bn6']); s6, t6 = float(s6[0]), float(t6[0])
    s7, t7 = _bn_affine(inputs['bn7']); s7, t7 = float(s7[0]), float(t7[0])

    # conv1 as dense position matmul, position-pair layout:
    # m1p[:, 128*pr : 128*pr+64]   -> ch of pos (y, 2j)   (pr = y*8+j)
    # m1p[:, 128*pr+64 : 128*pr+128] -> ch of pos (y, 2j+1)
    w1 = np.asarray(inputs['conv1_w'], np.float32)      # [64,1,3,3]
    m1 = np.zeros((128, NPOS, 64), np.float32)
    for p in range(NPOS):
        py, px = p // W, p % W
        for ky in range(3):
            for kx in range(3):
                iy, jx = py + ky - 1, px + kx - 1
                if 0 <= iy < H and 0 <= jx < W:
                    praw = 8 * jx + iy
                    m1[praw, p, :] += s1 * w1[:, 0, ky, kx]
    bias1 = (s1 * np.asarray(inputs['conv1_b'], np.float32) + t1)

    # fp8 activation scale for A2: safe L-inf bound on conv1 output
    xbn_max = abs(s0) * np.abs(np.asarray(inputs['x'], np.float32)).max() + abs(t0)
    a2_bound = np.abs(m1).sum(axis=0).max() * xbn_max + np.abs(bias1).max()
    sA2 = 96.0 / a2_bound

    m1p = np.zeros((128, NPAIR * 128), np.float32)
    for pr in range(NPAIR):
        y, j = pr // 8, pr % 8
        m1p[:, 128 * pr:128 * pr + 64] = sA2 * m1[:, y * W + 2 * j, :]
        m1p[:, 128 * pr + 64:128 * pr + 128] = sA2 * m1[:, y * W + 2 * j + 1, :]
    bias1 = sA2 * bias1

    # conv2 fp8 DoubleRow stationaries (SwInterleave packed).
    # Logical W[idx][k, slot, m]: k<64 ch of in-col A, k>=64 ch of in-col B
    # where (A,B) = (2j, 2j+1) for slot0, (2j+2, 2j+3) for slot1 (interior
    # idx=ky), and for wrap idx=3+ky: slot0=(0,1), slot1=(14,15).
    # m<64: out ch of col 2j'+1; m>=64: out ch of col (2j'+2)%16.
    w2r = np.asarray(inputs['conv2_w'], np.float32)     # [o, c, 3, 3]
    sW2 = 224.0 / np.abs(s2[:, None, None, None] * w2r).max()
    w2s = (sW2 * s2[:, None, None, None] * w2r)         # [o, c, ky, kx]
    w2n = np.zeros((128, 6 * 256), np.float32)
    for ky in range(3):
        for idx, (incols, outcols) in enumerate((
                (((0, 1), (2, 3)), (1, 2)),      # interior (rel cols)
                (((0, 1), (14, 15)), (15, 0)))):  # wrap (abs cols, slots (p0,p7))
            Wl = np.zeros((128, 2, 128), np.float32)
            for sl in range(2):
                for kk in range(2):
                    for mm in range(2):
                        ic, oc = incols[sl][kk], outcols[mm]
                        kx = ic - oc + 1
                        if 0 <= kx < 3:
                            Wl[64 * kk:64 * kk + 64, sl, 64 * mm:64 * mm + 64] = \
                                w2s[:, :, ky, kx].T
            # SwInterleave pack: mem[p, 2t+i] = Wl[p, i, 127-t]
            q = 3 * idx + ky
            dst = w2n[:, 256 * q:256 * q + 256]
            for t in range(128):
                dst[:, 2 * t] = Wl[:, 0, 127 - t]
                dst[:, 2 * t + 1] = Wl[:, 1, 127 - t]
    bias2 = sA2 * sW2 * (s2 * np.asarray(inputs['conv2_b'], np.float32) + t2)
    unsc = 1.0 / (sA2 * sW2)

    # pixel pair for pr' = y*8+j': (cols 2j'+1, (2j'+2)%16)
    def paircols(pr):
        y, j = pr // 8, pr % 8
        return y * W + 2 * j + 1, y * W + (2 * j + 2) % W

    # lc3/lc4 block-diagonal pixel-pair weights [128, 128] per pair.
    # swap_odd: odd pairs get (bo, be) block order — lc4 consumes the
    # swapped tmpB layout produced by lc3's crossed quad tiles.
    def lcbd(wname, s, swap_odd=False):
        wr = np.asarray(inputs[wname], np.float32)      # [o, c, h, w]
        out = np.zeros((128, NPAIR * 128), np.float32)
        for pr in range(NPAIR):
            pe, po = paircols(pr)
            be = s[None, :] * wr[:, :, pe // W, pe % W].T      # [c_in, o]
            bo = s[None, :] * wr[:, :, po // W, po % W].T
            if swap_odd and pr % 2 == 1:
                be, bo = bo, be
            out[0:64, 128 * pr:128 * pr + 64] = be
            out[64:128, 128 * pr + 64:128 * pr + 128] = bo
        return out
    w3bd = lcbd('lc3_w', s3 * unsc)
    w4bd = lcbd('lc4_w', s4, swap_odd=True)

    # fc5: rows of F are (k<64: ch k of even pix of pair; k>=64: ch k-64 of odd)
    # partition-major layout [128, pair*512] so DMA lines are 4KB contiguous
    fc5 = np.asarray(inputs['fc5_w'], np.float32)       # [512, 8192]
    fc5w = np.zeros((128, NPAIR * 512), np.float32)
    ch = np.arange(64)
    for pr in range(NPAIR):
        pe, po = paircols(pr)
        fc5w[0:64, 512 * pr:512 * (pr + 1)] = s5 * fc5[:, ch * 128 + pe].T
        fc5w[64:128, 512 * pr:512 * (pr + 1)] = s5 * fc5[:, ch * 128 + po].T
    bias5 = s5 * np.asarray(inputs['fc5_b'], np.float32) + t5   # [512]

    fc6 = np.asarray(inputs['fc6_w'], np.float32)       # [512, 512]
    fc6w = np.zeros((128, 16 * 128), np.float32)
    for jj in range(4):
        for m in range(4):
            blk = s6 * fc6[128 * m:128 * m + 128, 128 * jj:128 * jj + 128].T
            fc6w[:, (4 * jj + m) * 128:(4 * jj + m) * 128 + 128] = blk
    bias6 = s6 * np.asarray(inputs['fc6_b'], np.float32) + t6   # [512]

    fc7 = np.asarray(inputs['fc7_w'], np.float32)       # [128, 512]
    fc78w = np.zeros((128, 4 * 128 + NCLS), np.float32)
    for jj in range(4):
        fc78w[:, 128 * jj:128 * jj + 128] = s7 * fc7[:, 128 * jj:128 * jj + 128].T
    bias7 = s7 * np.asarray(inputs['fc7_b'], np.float32) + t7   # [128]
    fc8 = np.asarray(inputs['fc8_w'], np.float32)       # [8, 128]
    fc78w[:, 512:512 + NCLS] = fc8.T
    bias8 = np.asarray(inputs['fc8_b'], np.float32)     # [8]

    consts = np.zeros((128, 16), np.float32)
    consts[:, 0] = np.concatenate([bias1, bias1])
    consts[:, 1] = np.concatenate([bias2, bias2])
    consts[:, 2] = np.concatenate([t3, t3])
    consts[:, 3] = np.concatenate([t4, t4])
    for m in range(4):
        consts[:, 4 + m] = bias5[128 * m:128 * m + 128]
        consts[:, 8 + m] = bias6[128 * m:128 * m + 128]
    consts[:, 12] = bias7
    consts[:, 13] = s0
    consts[:, 14] = t0
    consts[0:NCLS, 15] = bias8

    return {
        'm1p': m1p.astype(bf), 'w2n': np.clip(w2n, -240, 240).astype(e4m3),
        'w3bd': w3bd.astype(bf),
        'w4bd': w4bd.astype(bf), 'fc5w': fc5w.astype(bf), 'fc6w': fc6w.astype(bf),
        'fc78w': fc78w.astype(bf), 'consts': consts,
    }


_cache = {}


def _get_nc(NB=1024, debug_taps=False):
    key = (NB, debug_taps)
    if key not in _cache:
        _cache[key] = build(NB, debug_taps)
    return _cache[key]


def _run(inputs, trace=False, tmpdir=None, debug_taps=False):
    from concourse.bass_utils import run_bass_kernel_spmd
    x = np.asarray(inputs['x'], np.float32)
    B = x.shape[0]
    NB = B // NCORES
    xf = x.reshape(B, 128)
    # bn0 folded + transposed on host: xts[pos, sample]
    s0, t0 = _bn_affine(inputs['bn0'])
    xts = np.ascontiguousarray((float(s0[0]) * xf + float(t0[0])).T
                               .astype(ml_dtypes.bfloat16))   # [128, B]
    w = prep_weights(inputs)
    nc = _get_nc(NB=NB, debug_taps=debug_taps)
    consts_bf = np.ascontiguousarray(w.pop('consts')).view(ml_dtypes.bfloat16)  # [128,32] raw bits
    m1p_full = w['m1p']
    in_maps = []
    for c in range(NCORES):
        m = dict(w)
        m['boot'] = np.ascontiguousarray(np.concatenate(
            [xts[:, c * NB:(c + 1) * NB], consts_bf, m1p_full[:, 0:1024]], axis=1))
        in_maps.append(m)
    kw = {}
    if trace:
        kw = dict(trace=True, tmpdir=tmpdir)
    res = run_bass_kernel_spmd(nc, in_maps, list(range(NCORES)), **kw)
    out = np.empty((B, NCLS), np.float32)
    for cc in range(NCORES):
        yc = np.asarray(res.results[cc]['y'], np.float32)   # [8, NB]
        out[cc * NB:(cc + 1) * NB] = yc.T
    return out, res


def kernel(**inputs):
    out, _ = _run(inputs)
    return out


def run_traced(inputs, tmpdir=None, debug_taps=False):
    return _run(inputs, trace=True, tmpdir=tmpdir, debug_taps=debug_taps)



# revision 43
# speedup vs baseline: 1.0070x; 1.0070x over previous
"""Trainium2 Bass kernel for nn_CapgMyoNet (dense CNN), 8-core data-parallel.

V2: position-parity layout redesign.
- A2 (conv1 out): partitions [0:64]=ch of even-x positions, [64:128]=ch of
  odd-x positions; cols = (y, xpair j, sample). All evictions dense 128-lane.
- conv1: M=128 matmuls (one per position pair), psum holds 2 pairs.
- conv2: K=128 matmuls using the parity layout; even-x outputs accumulate in
  psum[0:64] (tile (0,0)), odd-x in psum[64:128] (tile (0,64)); 12 weight
  mats (A/B even-class, C/D odd-class per ky).
- lc3/lc4: block-diagonal pixel-pair weights, K=128 M=128.
- fc5 accumulated on the fly; fc6/7/8 K-chunked as before.
All bn folded host-side; bf16 matmuls, fp32 accumulate.
"""
import numpy as np
import ml_dtypes

import concourse.bass as bass
import concourse.bacc as bacc
import concourse.mybir as mybir
import concourse.tile as tile

bf16 = mybir.dt.bfloat16
f32 = mybir.dt.float32
fp8 = mybir.dt.float8e4
DRSW = mybir.MatmulPerfMode.DoubleRowSwInterleave
e4m3 = ml_dtypes.float8_e4m3fn

H, W, C, NCLS = 8, 16, 64, 8
NPOS = H * W  # 128
NPAIR = 64    # 8 y rows x 8 x-pairs
EPS = 1e-5
NCORES = 8
PASSN = 256   # samples per conv pass
GROUP = 512   # samples per lc/fc5 group


def build(NB=1024, debug_taps=False):
    """Build the per-core bass program. NB = samples per core."""
    assert NB % GROUP == 0 and GROUP == 2 * PASSN
    NPASS = NB // PASSN
    NG = NB // GROUP
    NSPL = max(1, NB // 512)
    NCOLS = NB // NSPL

    nc = bacc.Bacc("TRN2", target_bir_lowering=False, debug=False)
    ev_ct = [0]

    def evict_split(out_ap, in_ap, bias_ap, n):
        """Latency-critical eviction: both halves in parallel on both engines."""
        h = n // 2
        nc.vector.tensor_scalar(out_ap[:, 0:h], in_ap[:, 0:h], bias_ap, 0.0,
                                mybir.AluOpType.add, mybir.AluOpType.max)
        nc.scalar.activation(out_ap[:, h:n], in_ap[:, h:n],
                             mybir.ActivationFunctionType.Relu, bias=bias_ap)

    def evict(out_ap, in_ap, bias_ap, relu=True):
        """Alternating-engine psum->sbuf eviction with bias (+relu)."""
        ev_ct[0] += 1
        if ev_ct[0] % 2 == 0:
            if relu:
                nc.scalar.activation(out_ap, in_ap,
                                     mybir.ActivationFunctionType.Relu,
                                     bias=bias_ap)
            else:
                nc.vector.tensor_scalar(out_ap, in_ap, bias_ap, None,
                                        mybir.AluOpType.add)
        else:
            if relu:
                nc.vector.tensor_scalar(out_ap, in_ap, bias_ap, 0.0,
                                        mybir.AluOpType.add,
                                        mybir.AluOpType.max)
            else:
                nc.vector.tensor_scalar(out_ap, in_ap, bias_ap, None,
                                        mybir.AluOpType.add)

    with tile.TileContext(nc) as tc:
        from contextlib import ExitStack
        es = ExitStack()
        with es:
            dram = es.enter_context(tc.tile_pool(name="dram", bufs=1, space="DRAM"))
            wp = es.enter_context(tc.tile_pool(name="wp", bufs=1))
            big = es.enter_context(tc.tile_pool(name="big", bufs=1))
            ring = es.enter_context(tc.tile_pool(name="ring", bufs=4))
            lcsb = es.enter_context(tc.tile_pool(name="lcsb", bufs=4))

            # ---------------- DRAM I/O ----------------
            boot_d = dram.tile([128, NB + 32 + 1024], bf16, kind="ExternalInput", name="boot", uniquify=False)
            m1p_d = dram.tile([128, NPAIR * 128], bf16, kind="ExternalInput", name="m1p", uniquify=False)
            w2n_d = dram.tile([128, 6 * 256], fp8, kind="ExternalInput", name="w2n", uniquify=False)
            w3bd_d = dram.tile([128, NPAIR * 128], bf16, kind="ExternalInput", name="w3bd", uniquify=False)
            w4bd_d = dram.tile([128, NPAIR * 128], bf16, kind="ExternalInput", name="w4bd", uniquify=False)
            fc5w_d = dram.tile([128, NPAIR * 512], bf16, kind="ExternalInput", name="fc5w", uniquify=False)
            fc6w_d = dram.tile([128, 16 * 128], bf16, kind="ExternalInput", name="fc6w", uniquify=False)
            fc78w_d = dram.tile([128, 4 * 128 + NCLS], bf16, kind="ExternalInput", name="fc78w", uniquify=False)

            y_d = dram.tile([NCLS, NB], f32, kind="ExternalOutput", name="y", uniquify=False)
            if debug_taps:
                dbg_a2_d = dram.tile([128, NPAIR * PASSN], f32, kind="ExternalOutput", name="dbg_a2", uniquify=False)
                dbg_a3_d = dram.tile([128, NPAIR * GROUP], f32, kind="ExternalOutput", name="dbg_a3", uniquify=False)
                dbg_f6_d = dram.tile([128, 4 * NB], f32, kind="ExternalOutput", name="dbg_f6", uniquify=False)

            # ---------------- persistent SBUF ----------------
            boot = wp.tile([128, NB + 32 + 1024], bf16, name="boot_sb")
            consts = boot[:, NB:NB + 32].bitcast(f32)
            B1 = consts[:, 0:1]; B2 = consts[:, 1:2]
            B3 = consts[:, 2:3]; B4 = consts[:, 3:4]
            B5 = [consts[:, 4 + m:5 + m] for m in range(4)]
            B6 = [consts[:, 8 + m:9 + m] for m in range(4)]
            B7 = consts[:, 12:13]
            S0 = consts[:, 13:14]; T0 = consts[:, 14:15]
            B8 = consts[0:NCLS, 15:16]

            w2n_sb = wp.tile([128, 6 * 256], fp8, name="w2n_sb")
            m1p_sb = wp.tile([128, NPAIR * 128 - 1024], bf16, name="m1p_sb")

            def m1pv(pr):
                if pr < 8:
                    return boot[:, NB + 32 + 128 * pr:NB + 32 + 128 * pr + 128]
                return m1p_sb[:, 128 * pr - 1024:128 * pr - 896]
            w3bd_sb = wp.tile([128, NPAIR * 128], bf16, name="w3bd_sb")
            w4bd_sb = wp.tile([128, NPAIR * 128], bf16, name="w4bd_sb")
            fc6w_sb = wp.tile([128, 16 * 128], bf16, name="fc6w_sb")
            fc78w_sb = wp.tile([128, 4 * 128 + NCLS], bf16, name="fc78w_sb")

            F6 = wp.tile([128, 4 * NB], bf16, name="F6")
            # A2/A3 split into quarter buffers (2 y-rows each) so consumers
            # wait on as few producer evictions as possible
            A2h = [big.tile([128, NPAIR * PASSN // 4], fp8, name=f"A2h{h}")
                   for h in range(4)]
            A3h = [big.tile([128, NPAIR * GROUP // 4], bf16, name=f"A3h{h}")
                   for h in range(4)]

            # ---------------- input/weight DMA issue ----------------
            # one "boot" DMA carries everything the first conv1 matmuls and
            # evicts need (xts, consts, m1p pairs 0-7): per-issue queue cost
            # is ~1.6us, so folding the three first-use loads into one issue
            # moves the first matmul ~5us earlier.  Everything else trails in
            # first-use order.  (issuing from scalar/gpsimd queues instead
            # was measured slower: it delays those engines' first evictions.)
            nc.sync.dma_start(out=boot[:], in_=boot_d[:])
            for q in range(4):
                a, b = 1024 + 1792 * q, min(1024 + 1792 * (q + 1), 8192)
                nc.sync.dma_start(out=m1p_sb[:, a - 1024:b - 1024],
                                  in_=m1p_d[:, a:b])
            nc.sync.dma_start(out=w2n_sb[:], in_=w2n_d[:])
            nc.sync.dma_start(out=w3bd_sb[:], in_=w3bd_d[:])
            nc.sync.dma_start(out=w4bd_sb[:], in_=w4bd_d[:])
            nc.sync.dma_start(out=fc6w_sb[:], in_=fc6w_d[:])
            nc.sync.dma_start(out=fc78w_sb[:], in_=fc78w_d[:])

            def xtsv(pi):
                return boot[:, PASSN * pi:PASSN * (pi + 1)]

            # A2 views: cols = (pair pr, s), pr = y*8 + j; quarters of 2 y-rows
            a2vh = [A2h[h][:].rearrange("q (y j s) -> q y j s", y=2, j=8, s=PASSN)
                    for h in range(4)]

            def a2row(r):
                return a2vh[r // 2][:, r % 2]
            a3vh = [A3h[h][:].rearrange("q (pr c) -> q pr c", pr=NPAIR // 4, c=GROUP)
                    for h in range(4)]
            f6v = F6[:].rearrange("q (m n) -> q m n", m=4, n=NB)
            F7 = wp.tile([128, 4 * NB], bf16, name="F7")
            F8 = wp.tile([128, NB], bf16, name="F8")
            y_sb = wp.tile([NCLS, NB], f32, name="y_sb")
            f7v = F7[:].rearrange("q (m n) -> q m n", m=4, n=NB)

            def fc6_stage(m, n0, w, pool):
                ps6 = pool.tile([128, 2 * PASSN], f32, name="psF", tag="psC2", bufs=4)
                for jj in range(4):
                    nc.tensor.matmul(ps6[:, 0:w],
                                     fc6w_sb[:, (4 * jj + m) * 128:(4 * jj + m) * 128 + 128],
                                     f6v[:, jj, n0:n0 + w],
                                     start=(jj == 0), stop=(jj == 3),
                                     skip_group_check=True)
                evict_split(f7v[:, m, n0:n0 + w], ps6[:, 0:w], B6[m], w)

            def fc7_stage(n0, w, pool):
                ps7 = pool.tile([128, 2 * PASSN], f32, name="psF", tag="psC2", bufs=4)
                for jj in range(4):
                    nc.tensor.matmul(ps7[:, 0:w],
                                     fc78w_sb[:, 128 * jj:128 * jj + 128],
                                     f7v[:, jj, n0:n0 + w],
                                     start=(jj == 0), stop=(jj == 3),
                                     skip_group_check=True)
                evict_split(F8[:, n0:n0 + w], ps7[:, 0:w], B7, w)

            def fc8_stage(n0, w, pool):
                ps8 = pool.tile([128, 2 * PASSN], f32, name="psF", tag="psC2", bufs=4)
                nc.tensor.matmul(ps8[0:NCLS, 0:w], fc78w_sb[:, 512:512 + NCLS],
                                 F8[:, n0:n0 + w], start=True, stop=True,
                                 skip_group_check=True)
                nc.vector.tensor_scalar(y_sb[:, n0:n0 + w], ps8[0:NCLS, 0:w], B8, None,
                                        mybir.AluOpType.add)
                if n0 + w == NB - 256:
                    nc.sync.dma_start(out=y_d[:, 0:n0 + w], in_=y_sb[:, 0:n0 + w])
                elif n0 + w == NB:
                    nc.sync.dma_start(out=y_d[:, n0:NB], in_=y_sb[:, n0:NB])

            def fc678(n, pool):
                """fc6/7/8 for one 512-sample group, split into half-group
                chunks so the serial evict->mm chains of the two halves
                overlap (matters for the final group = kernel tail)."""
                for h in range(2):
                    n0 = n * GROUP + h * (GROUP // 2)
                    for m in range(4):
                        fc6_stage(m, n0, GROUP // 2, pool)
                    fc7_stage(n0, GROUP // 2, pool)
                    fc8_stage(n0, GROUP // 2, pool)

            for g in range(NG):
                # ============ conv passes (2 per group) ============
                cpp_cm = tc.tile_pool(name=f"cps{g}", bufs=1, space="PSUM")
                cpp = cpp_cm.__enter__()
                for pb in range(2):
                    pi = 2 * g + pb
                    nb0 = pi * PASSN
                    c0 = pb * PASSN  # col offset within the group for A3
                    # ---- conv1: one M=128 matmul per position pair; psum
                    # tiles span 2 banks (4 pairs) so evictions amortize the
                    # ~120-cycle per-instruction PSUM-read bubble ----
                    if True:
                        c1pp = cpp
                        for t2 in range(NPAIR // 4):
                            psC1 = c1pp.tile([128, 4 * PASSN], f32, name="psC1", tag="psC1", bufs=2)
                            for sub in range(4):
                                pr = 4 * t2 + sub
                                nc.tensor.matmul(
                                    psC1[:, sub * PASSN:(sub + 1) * PASSN],
                                    m1pv(pr),
                                    xtsv(pi),
                                    start=True, stop=True,
                                    skip_group_check=True)
                            a2dst = A2h[t2 // 4][:, 4 * PASSN * (t2 % 4):4 * PASSN * (t2 % 4 + 1)]
                            if t2 % 16 in (0, 2, 4, 6, 8, 10, 11, 13, 15):
                                nc.scalar.activation(a2dst, psC1[:, :],
                                                     mybir.ActivationFunctionType.Relu,
                                                     bias=B1)
                            else:
                                nc.vector.tensor_scalar(a2dst, psC1[:, :], B1, 0.0,
                                                        mybir.AluOpType.add,
                                                        mybir.AluOpType.max)
                            # group-0 fc6/7/8 spread across pass-2's conv1 so
                            # its psum->sbuf chain hides under conv1 matmuls
                            if pi == 2 and t2 in (1, 3, 5, 9):
                                fc6_stage({1: 0, 3: 1, 5: 2, 9: 3}[t2], 0, GROUP, c1pp)
                            elif pi == 2 and t2 == 12:
                                fc7_stage(0, GROUP, c1pp)
                            elif pi == 2 and t2 == 15:
                                fc8_stage(0, GROUP, c1pp)
                            # (group-0 stages use full-width chunks)
                    # ---- conv2: fp8 DoubleRow (K=256 = two x-pairs) into A3 ----
                    # out pair j' = (cols 2j'+1, 2j'+2 mod 16); slots (pair j',
                    # pair j'+1) except j'=7 which uses slots (pair 0, pair 7)
                    # with a swapped wrap stationary.
                    w2v = w2n_sb[:].rearrange("p (q two m) -> p q two m",
                                              q=6, two=2, m=128)
                    if True:
                        c2pp = cpp
                        for yp in range(H):
                            kys = [ky for ky in range(3) if 0 <= yp + ky - 1 < H]
                            for jg in range(4):
                                psC2 = c2pp.tile([128, 512], f32, name="psC2", tag="psC2", bufs=4)
                                for u in range(2):
                                    jp = 2 * jg + u
                                    for i, ky in enumerate(kys):
                                        r = yp + ky - 1
                                        if jp < 7:
                                            rhs = a2row(r)[:, jp:jp + 2, :]
                                            lhsT = w2v[:, ky]
                                        else:
                                            rhs = a2row(r)[:, 0:8:7, :]
                                            lhsT = w2v[:, 3 + ky]
                                        nc.tensor.matmul(
                                            psC2[:, 256 * u:256 * u + 256],
                                            lhsT, rhs,
                                            start=(i == 0), stop=(i == len(kys) - 1),
                                            perf_mode=DRSW, skip_group_check=True)
                                ypl = yp % 2
                                evict(a3vh[yp // 2][:, 8 * ypl + 2 * jg:8 * ypl + 2 * jg + 2,
                                                    c0:c0 + PASSN],
                                      psC2[:].rearrange("q (j s) -> q j s", j=2, s=PASSN), B2)
                    if debug_taps and pi == NPASS - 1:
                        dbgt = wp.tile([128, NPAIR * PASSN], f32, name="dbg_a2sb")
                        nc.any.tensor_copy(dbgt[:, 0:NPAIR * PASSN // 2], A2h[0][:])
                        nc.sync.dma_start(out=dbg_a2_d[:], in_=dbgt[:])

                if debug_taps and g == 0:
                    dbgt3 = wp.tile([128, NPAIR * GROUP], f32, name="dbg_a3sb")
                    nc.any.tensor_copy(dbgt3[:], A3[:])
                    nc.sync.dma_start(out=dbg_a3_d[:], in_=dbgt3[:])

                cpp_cm.__exit__(None, None, None)
                # ============ lc3 + lc4 + fc5 over pixel pairs ============
                # lc3/lc4 run as 64x64 4-tile quads over pair-PAIRS (A=2P,
                # B=2P+1): pair A on diagonal tiles (0,0)/(64,64), pair B on
                # crossed tiles (0,64)/(64,0).  tmpB carries a swapped layout
                # (po-ch on partitions 0:64); lc4 un-swaps via crossed tiles
                # (w4bd has swapped blocks for odd pairs host-side), so F is
                # natural for fc5.  Slot pipeline keeps the two quads adjacent
                # (2 PE tiling-mode switches per slot, ~105ns each).
                with tc.tile_pool(name=f"lcps{g}", bufs=1, space="PSUM") as lcpp:
                    ps5 = [lcpp.tile([128, GROUP], f32, name=f"ps5_{m}", tag=f"ps5_{m}")
                           for m in range(4)]
                    tmps = {}
                    fjs = {}
                    wsts = {}

                    def fetch_fc5w(k):
                        wst = ring.tile([128, 2048], bf16, name="wst", tag="wst")
                        nc.sync.dma_start(out=wst[:], in_=fc5w_d[:, 2048 * k:2048 * (k + 1)])
                        wsts[k] = wst

                    def lc3_quad(P):
                        if P % 2 == 0 and P // 2 + 2 < NPAIR // 4:
                            fetch_fc5w(P // 2 + 2)
                        A, B = 2 * P, 2 * P + 1
                        aA = a3vh[A // 16][:, A % 16, :]
                        aB = a3vh[B // 16][:, B % 16, :]
                        psA = lcpp.tile([128, GROUP], f32, name="psA", tag="lcq3a")
                        psB = lcpp.tile([128, GROUP], f32, name="psB", tag="lcq3b")
                        nc.tensor.matmul(psA[0:64, :], w3bd_sb[0:64, 128 * A:128 * A + 64],
                                         aA[0:64], start=True, stop=True,
                                         tile_position=(0, 0), skip_group_check=True)
                        nc.tensor.matmul(psA[64:128, :], w3bd_sb[64:128, 128 * A + 64:128 * A + 128],
                                         aA[64:128], start=True, stop=True,
                                         tile_position=(64, 64), skip_group_check=True)
                        nc.tensor.matmul(psB[64:128, :], w3bd_sb[0:64, 128 * B:128 * B + 64],
                                         aB[0:64], start=True, stop=True,
                                         tile_position=(0, 64), skip_group_check=True)
                        nc.tensor.matmul(psB[0:64, :], w3bd_sb[64:128, 128 * B + 64:128 * B + 128],
                                         aB[64:128], start=True, stop=True,
                                         tile_position=(64, 0), skip_group_check=True)
                        tA = lcsb.tile([128, GROUP], bf16, name="tA", tag="tmp")
                        tB = lcsb.tile([128, GROUP], bf16, name="tB", tag="tmp")
                        evict(tA[:, :], psA[:, :], B3)
                        evict(tB[:, :], psB[:, :], B3)
                        tmps[A], tmps[B] = tA, tB

                    def lc4_quad(P):
                        A, B = 2 * P, 2 * P + 1
                        tA, tB = tmps.pop(A), tmps.pop(B)
                        psA = lcpp.tile([128, GROUP], f32, name="psA4", tag="lcq4a")
                        psB = lcpp.tile([128, GROUP], f32, name="psB4", tag="lcq4b")
                        nc.tensor.matmul(psA[0:64, :], w4bd_sb[0:64, 128 * A:128 * A + 64],
                                         tA[0:64, :], start=True, stop=True,
                                         tile_position=(0, 0), skip_group_check=True)
                        nc.tensor.matmul(psA[64:128, :], w4bd_sb[64:128, 128 * A + 64:128 * A + 128],
                                         tA[64:128, :], start=True, stop=True,
                                         tile_position=(64, 64), skip_group_check=True)
                        # pair B: tmpB swapped in, crossed tiles unswap the out
                        nc.tensor.matmul(psB[64:128, :], w4bd_sb[0:64, 128 * B:128 * B + 64],
                                         tB[0:64, :], start=True, stop=True,
                                         tile_position=(0, 64), skip_group_check=True)
                        nc.tensor.matmul(psB[0:64, :], w4bd_sb[64:128, 128 * B + 64:128 * B + 128],
                                         tB[64:128, :], start=True, stop=True,
                                         tile_position=(64, 0), skip_group_check=True)
                        FA = lcsb.tile([128, GROUP], bf16, name="FA", tag="Fj")
                        FB = lcsb.tile([128, GROUP], bf16, name="FB", tag="Fj")
                        evict(FA[:, :], psA[:, :], B4)
                        evict(FB[:, :], psB[:, :], B4)
                        fjs[A], fjs[B] = FA, FB

                    def fc5_stage(j):
                        Fj = fjs.pop(j)
                        wst = wsts[j // 4] if j % 4 != 3 else wsts.pop(j // 4)
                        for m in range(4):
                            nc.tensor.matmul(ps5[m][:, :],
                                             wst[:, 512 * (j % 4) + 128 * m:512 * (j % 4) + 128 * m + 128],
                                             Fj[:, :], start=(j == 0), stop=(j == NPAIR - 1))

                    fetch_fc5w(0); fetch_fc5w(1)
                    NQ = NPAIR // 2
                    # slot pipeline: [lc4(s-1), lc3(s)] adjacent in 64x64
                    # mode (2 switches per slot), then 2 fc5 stages.
                    lc3_quad(0)
                    for s in range(1, NQ + 2):
                        if s - 1 < NQ:
                            lc4_quad(s - 1)
                        if s < NQ:
                            lc3_quad(s)
                        if s - 2 >= 0:
                            fc5_stage(2 * (s - 2))
                            fc5_stage(2 * (s - 2) + 1)
                    for m in range(4):
                        evict_split(f6v[:, m, g * GROUP:(g + 1) * GROUP],
                                    ps5[m][:, :], B5[m], GROUP)

            # ============ fc6 / fc7 / fc8 for the last group ============
            if debug_taps:
                dbg6 = wp.tile([128, 4 * NB], f32, name="dbg_f6sb")
                nc.any.tensor_copy(dbg6[:], F6[:])
                nc.sync.dma_start(out=dbg_f6_d[:], in_=dbg6[:])

            with tc.tile_pool(name="fcps", bufs=3, space="PSUM") as fcpp:
                fc678(NG - 1, fcpp)

    nc.compile()
    return nc


# ---------------------------------------------------------------------------
# host-side weight preparation
# ---------------------------------------------------------------------------

def _bn_affine(p):
    g, b, m, v = p[0], p[1], p[2], p[3]
    s = g / np.sqrt(v + EPS)
    return s.astype(np.float32), (b - m * s).astype(np.float32)


def prep_weights(inputs, cast=True):
    bf = ml_dtypes.bfloat16 if cast else np.float32
    s0, t0 = _bn_affine(inputs['bn0']); s0, t0 = float(s0[0]), float(t0[0])
    s1, t1 = _bn_affine(inputs['bn1'])
    s2, t2 = _bn_affine(inputs['bn2'])
    s3, t3 = _bn_affine(inputs['bn3'])
    s4, t4 = _bn_affine(inputs['bn4'])
    s5, t5 = _bn_affine(inputs['bn5']); s5, t5 = float(s5[0]), float(t5[0])
    s6, t6 = _bn_affine(inputs['bn6']); s6, t6 = float(s6[0]), float(t6[0])
    s7, t7 = _bn_affine(inputs['bn7']); s7, t7 = float(s7[0]), float(t7[0])

    # conv1 as dense position matmul, position-pair layout:
    # m1p[:, 128*pr : 128*pr+64]   -> ch of pos (y, 2j)   (pr = y*8+j)
    # m1p[:, 128*pr+64 : 128*pr+128] -> ch of pos (y, 2j+1)
    w1 = np.asarray(inputs['conv1_w'], np.float32)      # [64,1,3,3]
    m1 = np.zeros((128, NPOS, 64), np.float32)
    for p in range(NPOS):
        py, px = p // W, p % W
        for ky in range(3):
            for kx in range(3):
                iy, jx = py + ky - 1, px + kx - 1
                if 0 <= iy < H and 0 <= jx < W:
                    praw = 8 * jx + iy
                    m1[praw, p, :] += s1 * w1[:, 0, ky, kx]
    bias1 = (s1 * np.asarray(inputs['conv1_b'], np.float32) + t1)

    # fp8 activation scale for A2: safe L-inf bound on conv1 output
    xbn_max = abs(s0) * np.abs(np.asarray(inputs['x'], np.float32)).max() + abs(t0)
    a2_bound = np.abs(m1).sum(axis=0).max() * xbn_max + np.abs(bias1).max()
    sA2 = 96.0 / a2_bound

    m1p = np.zeros((128, NPAIR * 128), np.float32)
    for pr in range(NPAIR):
        y, j = pr // 8, pr % 8
        m1p[:, 128 * pr:128 * pr + 64] = sA2 * m1[:, y * W + 2 * j, :]
        m1p[:, 128 * pr + 64:128 * pr + 128] = sA2 * m1[:, y * W + 2 * j + 1, :]
    bias1 = sA2 * bias1

    # conv2 fp8 DoubleRow stationaries (SwInterleave packed).
    # Logical W[idx][k, slot, m]: k<64 ch of in-col A, k>=64 ch of in-col B
    # where (A,B) = (2j, 2j+1) for slot0, (2j+2, 2j+3) for slot1 (interior
    # idx=ky), and for wrap idx=3+ky: slot0=(0,1), slot1=(14,15).
    # m<64: out ch of col 2j'+1; m>=64: out ch of col (2j'+2)%16.
    w2r = np.asarray(inputs['conv2_w'], np.float32)     # [o, c, 3, 3]
    sW2 = 224.0 / np.abs(s2[:, None, None, None] * w2r).max()
    w2s = (sW2 * s2[:, None, None, None] * w2r)         # [o, c, ky, kx]
    w2n = np.zeros((128, 6 * 256), np.float32)
    for ky in range(3):
        for idx, (incols, outcols) in enumerate((
                (((0, 1), (2, 3)), (1, 2)),      # interior (rel cols)
                (((0, 1), (14, 15)), (15, 0)))):  # wrap (abs cols, slots (p0,p7))
            Wl = np.zeros((128, 2, 128), np.float32)
            for sl in range(2):
                for kk in range(2):
                    for mm in range(2):
                        ic, oc = incols[sl][kk], outcols[mm]
                        kx = ic - oc + 1
                        if 0 <= kx < 3:
                            Wl[64 * kk:64 * kk + 64, sl, 64 * mm:64 * mm + 64] = \
                                w2s[:, :, ky, kx].T
            # SwInterleave pack: mem[p, 2t+i] = Wl[p, i, 127-t]
            q = 3 * idx + ky
            dst = w2n[:, 256 * q:256 * q + 256]
            for t in range(128):
                dst[:, 2 * t] = Wl[:, 0, 127 - t]
                dst[:, 2 * t + 1] = Wl[:, 1, 127 - t]
    bias2 = sA2 * sW2 * (s2 * np.asarray(inputs['conv2_b'], np.float32) + t2)
    unsc = 1.0 / (sA2 * sW2)

    # pixel pair for pr' = y*8+j': (cols 2j'+1, (2j'+2)%16)
    def paircols(pr):
        y, j = pr // 8, pr % 8
        return y * W + 2 * j + 1, y * W + (2 * j + 2) % W

    # lc3/lc4 block-diagonal pixel-pair weights [128, 128] per pair.
    # swap_odd: odd pairs get (bo, be) block order — lc4 consumes the
    # swapped tmpB layout produced by lc3's crossed quad tiles.
    def lcbd(wname, s, swap_odd=False):
        wr = np.asarray(inputs[wname], np.float32)      # [o, c, h, w]
        out = np.zeros((128, NPAIR * 128), np.float32)
        for pr in range(NPAIR):
            pe, po = paircols(pr)
            be = s[None, :] * wr[:, :, pe // W, pe % W].T      # [c_in, o]
            bo = s[None, :] * wr[:, :, po // W, po % W].T
            if swap_odd and pr % 2 == 1:
                be, bo = bo, be
            out[0:64, 128 * pr:128 * pr + 64] = be
            out[64:128, 128 * pr + 64:128 * pr + 128] = bo
        return out
    w3bd = lcbd('lc3_w', s3 * unsc)
    w4bd = lcbd('lc4_w', s4, swap_odd=True)

    # fc5: rows of F are (k<64: ch k of even pix of pair; k>=64: ch k-64 of odd)
    # partition-major layout [128, pair*512] so DMA lines are 4KB contiguous
    fc5 = np.asarray(inputs['fc5_w'], np.float32)       # [512, 8192]
    fc5w = np.zeros((128, NPAIR * 512), np.float32)
    ch = np.arange(64)
    for pr in range(NPAIR):
        pe, po = paircols(pr)
        fc5w[0:64, 512 * pr:512 * (pr + 1)] = s5 * fc5[:, ch * 128 + pe].T
        fc5w[64:128, 512 * pr:512 * (pr + 1)] = s5 * fc5[:, ch * 128 + po].T
    bias5 = s5 * np.asarray(inputs['fc5_b'], np.float32) + t5   # [512]

    fc6 = np.asarray(inputs['fc6_w'], np.float32)       # [512, 512]
    fc6w = np.zeros((128, 16 * 128), np.float32)
    for jj in range(4):
        for m in range(4):
            blk = s6 * fc6[128 * m:128 * m + 128, 128 * jj:128 * jj + 128].T
            fc6w[:, (4 * jj + m) * 128:(4 * jj + m) * 128 + 128] = blk
    bias6 = s6 * np.asarray(inputs['fc6_b'], np.float32) + t6   # [512]

    fc7 = np.asarray(inputs['fc7_w'], np.float32)       # [128, 512]
    fc78w = np.zeros((128, 4 * 128 + NCLS), np.float32)
    for jj in range(4):
        fc78w[:, 128 * jj:128 * jj + 128] = s7 * fc7[:, 128 * jj:128 * jj + 128].T
    bias7 = s7 * np.asarray(inputs['fc7_b'], np.float32) + t7   # [128]
    fc8 = np.asarray(inputs['fc8_w'], np.float32)       # [8, 128]
    fc78w[:, 512:512 + NCLS] = fc8.T
    bias8 = np.asarray(inputs['fc8_b'], np.float32)     # [8]

    consts = np.zeros((128, 16), np.float32)
    consts[:, 0] = np.concatenate([bias1, bias1])
    consts[:, 1] = np.concatenate([bias2, bias2])
    consts[:, 2] = np.concatenate([t3, t3])
    consts[:, 3] = np.concatenate([t4, t4])
    for m in range(4):
        consts[:, 4 + m] = bias5[128 * m:128 * m + 128]
        consts[:, 8 + m] = bias6[128 * m:128 * m + 128]
    consts[:, 12] = bias7
    consts[:, 13] = s0
    consts[:, 14] = t0
    consts[0:NCLS, 15] = bias8

    return {
        'm1p': m1p.astype(bf), 'w2n': np.clip(w2n, -240, 240).astype(e4m3),
        'w3bd': w3bd.astype(bf),
        'w4bd': w4bd.astype(bf), 'fc5w': fc5w.astype(bf), 'fc6w': fc6w.astype(bf),
        'fc78w': fc78w.astype(bf), 'consts': consts,
    }


_cache = {}


def _get_nc(NB=1024, debug_taps=False):
    key = (NB, debug_taps)
    if key not in _cache:
        _cache[key] = build(NB, debug_taps)
    return _cache[key]


def _run(inputs, trace=False, tmpdir=None, debug_taps=False):
    from concourse.bass_utils import run_bass_kernel_spmd
    x = np.asarray(inputs['x'], np.float32)
    B = x.shape[0]
    NB = B // NCORES
    xf = x.reshape(B, 128)
    # bn0 folded + transposed on host: xts[pos, sample]
    s0, t0 = _bn_affine(inputs['bn0'])
    xts = np.ascontiguousarray((float(s0[0]) * xf + float(t0[0])).T
                               .astype(ml_dtypes.bfloat16))   # [128, B]
    w = prep_weights(inputs)
    nc = _get_nc(NB=NB, debug_taps=debug_taps)
    consts_bf = np.ascontiguousarray(w.pop('consts')).view(ml_dtypes.bfloat16)  # [128,32] raw bits
    m1p_full = w['m1p']
    in_maps = []
    for c in range(NCORES):
        m = dict(w)
        m['boot'] = np.ascontiguousarray(np.concatenate(
            [xts[:, c * NB:(c + 1) * NB], consts_bf, m1p_full[:, 0:1024]], axis=1))
        in_maps.append(m)
    kw = {}
    if trace:
        kw = dict(trace=True, tmpdir=tmpdir)
    res = run_bass_kernel_spmd(nc, in_maps, list(range(NCORES)), **kw)
    out = np.empty((B, NCLS), np.float32)
    for cc in range(NCORES):
        yc = np.asarray(res.results[cc]['y'], np.float32)   # [8, NB]
        out[cc * NB:(cc + 1) * NB] = yc.T
    return out, res


def kernel(**inputs):
    out, _ = _run(inputs)
    return out


def run_traced(inputs, tmpdir=None, debug_taps=False):
    return _run(inputs, trace=True, tmpdir=tmpdir, debug_taps=debug_taps)

